# revision 19
# baseline (speedup 1.0000x reference)
"""MultiHeadAttention (softmax over heads) Trainium2 kernel.

Math (per batch b):
  k[t, o]   = value[b, t, :] @ conv_w[o, :, 0] + conv_b[o]
  s[h, q, t] = sum_d query[b, q, h*64+d] * k[t, h*64+d]
  w[h, q, t] = softmax over h of (s/8)              (legacy implicit dim=1)
  out[b, q, h*64+d] = sum_t w[h, q, t] * value[b, t, d]

Key identity: s[h,q,t] = qW[h,q,:] . value[t,:] + qb[h,q] where
  qW[h,q,i] = sum_d query[q, h*64+d] * conv_w[h*64+d, i]
  qb[h,q]   = sum_d query[q, h*64+d] * conv_b[h*64+d]
so all 8 heads' scores share the same rhs (value) with contraction over the
64 value features, and k is never materialized. The bias is folded in by
augmenting the contraction with a ones row (vT1 row 64).

Layout: scores live t-on-partitions as s[t, h, q] so the h-softmax is a
free-dim segmented reduction and stage B (ctxT[d, (h,q)] += v.T w) consumes
the weights directly. Final PE transposes produce out[q, h*64+d].

Sharding: data-parallel over (batch, query-rows): core c handles batch c//4,
query rows (c%4)*512 ... +512. No collectives.
"""

import sys

sys.path.insert(0, "/opt/trn_rl_repo")

import numpy as np
import ml_dtypes

import concourse.bass as bass
import concourse.bacc as bacc
import concourse.tile as tile
from concourse import mybir
from concourse.bass_utils import run_bass_kernel_spmd
from concourse.masks import make_identity

N_CORES = 8
B, TQ, TV, D, H, DH = 2, 2048, 2048, 512, 8, 64
QPC = (B * TQ) // N_CORES  # 512 query rows per core
QB = 128                   # q-block
NQB = QPC // QB            # 4
TCH = 128                  # t-chunk
NTC = TV // TCH            # 16
GRP = 4                    # t-chunks per softmax batch
NGRP = NTC // GRP

F32 = mybir.dt.float32
F32R = mybir.dt.float32r
BF16 = mybir.dt.bfloat16
SA_F32R = True  # fp32r score path (set before build_nc/_prep_inputs)

_CACHE = {}
BUILD_CFG = dict(tail_g=2, ctx_act=True, o_act=True, gp_cast=True)


def build_nc(reps=1, gp_offload=False, grp=GRP, eb=4, tb=4, db=6, wb=4, sb=3, gp_cast=False, qw_act=True, ctx_act=False, o_act=False, tail_g=1, gp_l2=False, gp_mulh=False, sa_f32r=None):
    if sa_f32r is None:
        sa_f32r = SA_F32R
    SDT = F32R if sa_f32r else BF16     # score-path storage/matmul dtype
    mm_cast = lambda ap: ap
    nc = bacc.Bacc("TRN2", target_bir_lowering=False, debug=False,
                   num_devices=N_CORES)

    # Per-core inputs (host-prepped layouts, bf16 matmul operands).
    # qTp[d, h, q] = query[q, h*64+d]
    qTp = nc.dram_tensor("qTp", [DH, H * QPC], SDT, kind="ExternalInput")
    # vT1[i, t] = value[t, i] for i<64; row 64 = ones
    vT1 = nc.dram_tensor("vT1", [DH + 1, TV], SDT, kind="ExternalInput")
    # vP[p, c, d] = value[c*128 + p, d]
    vP = nc.dram_tensor("vP", [TCH, NTC * DH], BF16, kind="ExternalInput")
    # wA[d, h, i] = conv_w[h*64+d, i, 0] (i<64); wA[d, h, 64] = conv_b[h*64+d]
    wA = nc.dram_tensor("wA", [DH, H * (DH + 1)], SDT, kind="ExternalInput")
    out = nc.dram_tensor("out", [QPC, D], F32, kind="ExternalOutput")

    with tile.TileContext(nc) as tc:
        with (
            tc.tile_pool(name="consts", bufs=1) as consts,
            tc.tile_pool(name="qwt", bufs=1) as qwt_pool,
            tc.tile_pool(name="vals", bufs=1) as vals,
        ):
            # ---- constants / weights ----
            ident = consts.tile([2 * DH, 2 * DH], F32)
            make_identity(nc, ident)

            w_sb = consts.tile([DH, H, DH + 1], SDT)
            nc.sync.dma_start(out=w_sb, in_=wA.rearrange("d (h i) -> d h i", h=H))

            vT_sb = vals.tile([DH + 1, TV], SDT)
            nc.sync.dma_start(out=vT_sb, in_=vT1[:, :])

            # ---- qWT' computation: [65, H, QPC] in two h-halves ----
            qTp3 = qTp.rearrange("d (h q) -> d h q", h=H)
            qwt_halves = []
            with (
                tc.tile_pool(name="qt_in", bufs=2) as qt_in,
                tc.tile_pool(name="qw_ps", bufs=2, space="PSUM") as qw_ps,
            ):
                for half in (0, 1):
                    qt_sb = qt_in.tile([DH, 4, QPC], SDT)
                    eng = nc.gpsimd if half == 0 else nc.scalar
                    eng.dma_start(out=qt_sb, in_=qTp3[:, half * 4:half * 4 + 4, :])
                    qwt_h = qwt_pool.tile([DH + 1, 4, QPC], SDT,
                                          tag=f"qwt{half}")
                    for hh in range(4):
                        h = half * 4 + hh
                        ps = qw_ps.tile([DH + 1, QPC], F32)
                        nc.tensor.matmul(ps, lhsT=mm_cast(w_sb[:, h, :]),
                                         rhs=mm_cast(qt_sb[:, hh, :]),
                                         start=True, stop=True)
                        if qw_act:
                            nc.scalar.copy(qwt_h[:, hh, :], ps)
                        else:
                            nc.vector.tensor_copy(qwt_h[:, hh, :], ps)
                    qwt_halves.append(qwt_h)

            v_sb = vals.tile([TCH, NTC, DH], BF16)
            nc.sync.dma_start(out=v_sb, in_=vP.rearrange("p (c d) -> p c d", c=NTC))

            # ---- main loop ----
            with (
                tc.tile_pool(name="s_ps", bufs=sb, space="PSUM") as s_ps_pool,
                tc.tile_pool(name="ctx_ps", bufs=2, space="PSUM") as ctx_ps_pool,
                tc.tile_pool(name="e_sb", bufs=eb) as e_pool,
                tc.tile_pool(name="tr_sb", bufs=tb) as tr_pool,
                tc.tile_pool(name="d_sb", bufs=db) as d_pool,
                tc.tile_pool(name="w_sb2", bufs=wb) as wt_pool,
                tc.tile_pool(name="ctx_sb", bufs=2) as ctx_sb_pool,
            ):
                import contextlib
                rep_ctx = tc.For_i(0, reps, 1) if reps > 1 else contextlib.nullcontext()
                prev_tail = [None]
                with rep_ctx:
                    for qb in range(NQB):
                        q0 = qb * QB
                        ctx_ps = ctx_ps_pool.tile([2 * DH, H, QB // 2], F32, tag="ctx")
                        ngrp = NTC // grp

                        def emit_stage_b(g, wt, ctx_ps=ctx_ps):
                            for ci in range(grp):
                                t = g * grp + ci
                                for sub in (0, 1):
                                    nc.tensor.matmul(
                                        ctx_ps[sub * DH:(sub + 1) * DH, :, :],
                                        lhsT=v_sb[:, t, :],
                                        rhs=wt[:, ci, :,
                                               sub * (QB // 2):(sub + 1) * (QB // 2)],
                                        start=(t == 0), stop=(t == NTC - 1),
                                        tile_position=(0, sub * DH),
                                        skip_group_check=True)

                        def emit_tail(qb_, ctx_ps_):
                            q0_ = qb_ * QB
                            ctx_sb = ctx_sb_pool.tile([2 * DH, H, QB // 2], F32)
                            if ctx_act:
                                nc.scalar.copy(ctx_sb, ctx_ps_)
                            else:
                                nc.vector.tensor_copy(ctx_sb, ctx_ps_)
                            o_ps = s_ps_pool.tile([DH, 2, H, DH], F32, tag="s")
                            for h in range(H):
                                for sub in (0, 1):
                                    nc.tensor.transpose(
                                        o_ps[:, sub, h, :],
                                        ctx_sb[sub * DH:(sub + 1) * DH, h, :],
                                        ident[sub * DH:(sub + 1) * DH,
                                              sub * DH:(sub + 1) * DH])
                            o_sb = ctx_sb_pool.tile([DH, 2, H, DH], F32, tag="o_sb")
                            if o_act:
                                nc.scalar.copy(o_sb, o_ps)
                            else:
                                nc.vector.tensor_copy(o_sb, o_ps)
                            out_ap = bass.AP(
                                tensor=out, offset=q0_ * D,
                                ap=[[D, DH], [DH * D, 2], [1, D]],
                            )
                            nc.sync.dma_start(out=out_ap, in_=o_sb)

                        pending = None  # (g, wt) whose stage-B is deferred
                        for g in range(ngrp):
                            e = e_pool.tile([TCH, grp, H, QB], BF16)
                            wt = wt_pool.tile([TCH, grp, H, QB], BF16)
                            for ci in range(grp):
                                t = g * grp + ci
                                s_ps = s_ps_pool.tile([TCH, H, QB], F32,
                                                      tag="s")
                                lhsT = vT_sb[:, t * TCH:(t + 1) * TCH]
                                nc.tensor.matmul(
                                    s_ps[:, 0:4, :], lhsT=mm_cast(lhsT),
                                    rhs=mm_cast(qwt_halves[0][:, :, q0:q0 + QB]),
                                    start=True, stop=True)
                                nc.tensor.matmul(
                                    s_ps[:, 4:8, :], lhsT=mm_cast(lhsT),
                                    rhs=mm_cast(qwt_halves[1][:, :, q0:q0 + QB]),
                                    start=True, stop=True)
                                nc.scalar.activation(
                                    out=e[:, ci, :, :], in_=s_ps,
                                    func=mybir.ActivationFunctionType.Exp,
                                    scale=0.125)

                            if g == tail_g and prev_tail[0] is not None:
                                emit_tail(*prev_tail[0])
                                prev_tail[0] = None
                            if pending is not None:
                                emit_stage_b(*pending)

                            # softmax over h, batched over the GRP chunks
                            gv = nc.gpsimd if gp_offload else nc.vector
                            t1 = tr_pool.tile([TCH, grp, 4, QB], BF16, tag="t1")
                            gv.tensor_add(t1, e[:, :, 0:4, :], e[:, :, 4:8, :])
                            t2 = tr_pool.tile([TCH, grp, 2, QB], BF16, tag="t2")
                            (nc.gpsimd if gp_l2 else nc.vector).tensor_add(
                                t2, t1[:, :, 0:2, :], t1[:, :, 2:4, :])
                            dsum = d_pool.tile([TCH, grp, QB], F32, tag="dsum")
                            nc.vector.tensor_add(dsum, t2[:, :, 0, :],
                                                 t2[:, :, 1, :])
                            r32 = d_pool.tile([TCH, grp, QB], F32, tag="r32")
                            nc.vector.reciprocal_approx_fast(out=r32, in_=dsum)
                            r16 = d_pool.tile([TCH, grp, QB], BF16, tag="r16")
                            (nc.gpsimd if gp_cast else nc.vector).tensor_copy(r16, r32)

                            r_bcast = bass.AP(
                                tensor=r16.tensor, offset=r16.offset,
                                ap=[r16.ap[0], r16.ap[1], [0, H], r16.ap[2]],
                            )
                            if gp_mulh:
                                r_bcast4 = bass.AP(
                                    tensor=r16.tensor, offset=r16.offset,
                                    ap=[r16.ap[0], r16.ap[1], [0, 4], r16.ap[2]],
                                )
                                nc.vector.tensor_mul(wt[:, :, 0:4, :],
                                                     e[:, :, 0:4, :], r_bcast4)
                                nc.gpsimd.tensor_mul(wt[:, :, 4:8, :],
                                                     e[:, :, 4:8, :], r_bcast4)
                            else:
                                nc.vector.tensor_mul(wt, e, r_bcast)
                            pending = (g, wt)

                        emit_stage_b(*pending)
                        prev_tail[0] = (qb, ctx_ps)
                    if prev_tail[0] is not None:
                        emit_tail(*prev_tail[0])
                        prev_tail[0] = None
    nc.compile()
    return nc


def _prep_inputs(query, value, conv_w, conv_b):
    """Build the 8 per-core input maps (host-side sharding + layout)."""
    sdt = np.float32 if SA_F32R else ml_dtypes.bfloat16
    W = conv_w[:, :, 0]  # [512, 64]
    w_aug = np.zeros((DH, H, DH + 1), np.float32)
    w_aug[:, :, :DH] = W.reshape(H, DH, DH).transpose(1, 0, 2)
    w_aug[:, :, DH] = conv_b.reshape(H, DH).T
    w_aug = w_aug.reshape(DH, H * (DH + 1)).astype(sdt)

    per_batch = {}
    for b in range(B):
        vT1 = np.concatenate(
            [value[b].T, np.ones((1, TV), np.float32)], axis=0
        ).astype(sdt)
        vPb = np.ascontiguousarray(
            value[b].reshape(NTC, TCH, DH).transpose(1, 0, 2)
        ).reshape(TCH, NTC * DH).astype(ml_dtypes.bfloat16)
        per_batch[b] = (np.ascontiguousarray(vT1), np.ascontiguousarray(vPb))

    in_maps = []
    for c in range(N_CORES):
        b = c // (N_CORES // B)
        qs = (c % (N_CORES // B)) * QPC
        # qTp[d, h, q] = query[b, qs+q, h*64+d]
        qTp = np.ascontiguousarray(
            query[b, qs:qs + QPC, :].reshape(QPC, H, DH).transpose(2, 1, 0)
        ).reshape(DH, H * QPC).astype(sdt)
        vT1, vPb = per_batch[b]
        in_maps.append({
            "qTp": np.ascontiguousarray(qTp),
            "vT1": vT1,
            "vP": vPb,
            "wA": w_aug,
        })
    return in_maps


def kernel(query, value, conv_w, conv_b, trace=False, **bench_kwargs):
    query = np.asarray(query, np.float32)
    value = np.asarray(value, np.float32)
    conv_w = np.asarray(conv_w, np.float32)
    conv_b = np.asarray(conv_b, np.float32)

    if "nc" not in _CACHE:
        _CACHE["nc"] = build_nc(**BUILD_CFG)
    nc = _CACHE["nc"]

    in_maps = _prep_inputs(query, value, conv_w, conv_b)
    res = run_bass_kernel_spmd(nc, in_maps, core_ids=list(range(N_CORES)),
                               trace=trace, **bench_kwargs)

    out = np.empty((B, TQ, D), np.float32)
    for c in range(N_CORES):
        b = c // (N_CORES // B)
        qs = (c % (N_CORES // B)) * QPC
        out[b, qs:qs + QPC, :] = res.results[c]["out"]
    if trace:
        return out, res
    return out


# revision 20
# speedup vs baseline: 1.0939x; 1.0939x over previous
"""MultiHeadAttention (softmax over heads) Trainium2 kernel.

Math (per batch b):
  k[t, o]   = value[b, t, :] @ conv_w[o, :, 0] + conv_b[o]
  s[h, q, t] = sum_d query[b, q, h*64+d] * k[t, h*64+d]
  w[h, q, t] = softmax over h of (s/8)              (legacy implicit dim=1)
  out[b, q, h*64+d] = sum_t w[h, q, t] * value[b, t, d]

Key identity: s[h,q,t] = qW[h,q,:] . value[t,:] + qb[h,q] where
  qW[h,q,i] = sum_d query[q, h*64+d] * conv_w[h*64+d, i]
  qb[h,q]   = sum_d query[q, h*64+d] * conv_b[h*64+d]
so all 8 heads' scores share the same rhs (value) with contraction over the
64 value features, and k is never materialized. The bias is folded in by
augmenting the contraction with a ones row (vT1 row 64).

Layout: scores live t-on-partitions as s[t, h, q] so the h-softmax is a
free-dim segmented reduction and stage B (ctxT[d, (h,q)] += v.T w) consumes
the weights directly. Final PE transposes produce out[q, h*64+d].

Sharding: data-parallel over (batch, query-rows): core c handles batch c//4,
query rows (c%4)*512 ... +512. No collectives.
"""

import sys

sys.path.insert(0, "/opt/trn_rl_repo")

import numpy as np
import ml_dtypes

import concourse.bass as bass
import concourse.bacc as bacc
import concourse.tile as tile
from concourse import mybir
from concourse.bass_utils import run_bass_kernel_spmd
from concourse.masks import make_identity

N_CORES = 8
B, TQ, TV, D, H, DH = 2, 2048, 2048, 512, 8, 64
QPC = (B * TQ) // N_CORES  # 512 query rows per core
QB = 128                   # q-block
NQB = QPC // QB            # 4
TCH = 128                  # t-chunk
NTC = TV // TCH            # 16
GRP = 4                    # t-chunks per softmax batch
NGRP = NTC // GRP

F32 = mybir.dt.float32
F32R = mybir.dt.float32r
BF16 = mybir.dt.bfloat16
SA_F32R = True  # fp32r score path (set before build_nc/_prep_inputs)

_CACHE = {}
BUILD_CFG = dict(tail_g=2, ctx_act=True, o_act=True, gp_cast=False)


def build_nc(reps=1, gp_offload=False, grp=GRP, eb=4, tb=4, db=6, wb=4, sb=3, gp_cast=False, qw_act=True, ctx_act=False, o_act=False, tail_g=1, gp_l2=False, gp_mulh=False, sa_f32r=None):
    if sa_f32r is None:
        sa_f32r = SA_F32R
    SDT = F32R if sa_f32r else BF16     # score-path storage/matmul dtype
    mm_cast = lambda ap: ap
    nc = bacc.Bacc("TRN2", target_bir_lowering=False, debug=False,
                   num_devices=N_CORES)

    # Per-core inputs (host-prepped layouts, bf16 matmul operands).
    # qTp[d, h, q] = query[q, h*64+d]
    qTp = nc.dram_tensor("qTp", [DH, H * QPC], SDT, kind="ExternalInput")
    # vT1[i, t] = value[t, i] for i<64; row 64 = ones
    vT1 = nc.dram_tensor("vT1", [DH + 1, TV], SDT, kind="ExternalInput")
    # vP[p, c, d] = value[c*128 + p, d]
    vP = nc.dram_tensor("vP", [TCH, NTC * DH], BF16, kind="ExternalInput")
    # wA[d, h, i] = conv_w[h*64+d, i, 0] (i<64); wA[d, h, 64] = conv_b[h*64+d]
    wA = nc.dram_tensor("wA", [DH, H * (DH + 1)], SDT, kind="ExternalInput")
    out = nc.dram_tensor("out", [QPC, D], F32, kind="ExternalOutput")

    with tile.TileContext(nc) as tc:
        with (
            tc.tile_pool(name="consts", bufs=1) as consts,
            tc.tile_pool(name="qwt", bufs=1) as qwt_pool,
            tc.tile_pool(name="vals", bufs=1) as vals,
        ):
            # ---- constants / weights ----
            ident = consts.tile([2 * DH, 2 * DH], F32)
            make_identity(nc, ident)

            w_sb = consts.tile([DH, H, DH + 1], SDT)
            nc.sync.dma_start(out=w_sb, in_=wA.rearrange("d (h i) -> d h i", h=H))

            vT_sb = vals.tile([DH + 1, TV], SDT)
            nc.sync.dma_start(out=vT_sb, in_=vT1[:, :])

            # ---- qWT' computation: [65, H, QPC] in two h-halves ----
            qTp3 = qTp.rearrange("d (h q) -> d h q", h=H)
            qwt_halves = []
            with (
                tc.tile_pool(name="qt_in", bufs=2) as qt_in,
                tc.tile_pool(name="qw_ps", bufs=2, space="PSUM") as qw_ps,
            ):
                for half in (0, 1):
                    qt_sb = qt_in.tile([DH, 4, QPC], SDT)
                    eng = nc.gpsimd if half == 0 else nc.scalar
                    eng.dma_start(out=qt_sb, in_=qTp3[:, half * 4:half * 4 + 4, :])
                    qwt_h = qwt_pool.tile([DH + 1, 4, QPC], SDT,
                                          tag=f"qwt{half}")
                    for hh in range(4):
                        h = half * 4 + hh
                        ps = qw_ps.tile([DH + 1, QPC], F32)
                        nc.tensor.matmul(ps, lhsT=mm_cast(w_sb[:, h, :]),
                                         rhs=mm_cast(qt_sb[:, hh, :]),
                                         start=True, stop=True)
                        if qw_act:
                            nc.scalar.copy(qwt_h[:, hh, :], ps)
                        else:
                            nc.vector.tensor_copy(qwt_h[:, hh, :], ps)
                    qwt_halves.append(qwt_h)

            v_sb = vals.tile([TCH, NTC, DH], BF16)
            nc.sync.dma_start(out=v_sb, in_=vP.rearrange("p (c d) -> p c d", c=NTC))

            # ---- main loop ----
            with (
                tc.tile_pool(name="s_ps", bufs=sb, space="PSUM") as s_ps_pool,
                tc.tile_pool(name="ctx_ps", bufs=2, space="PSUM") as ctx_ps_pool,
                tc.tile_pool(name="e_sb", bufs=eb) as e_pool,
                tc.tile_pool(name="tr_sb", bufs=tb) as tr_pool,
                tc.tile_pool(name="d_sb", bufs=db) as d_pool,
                tc.tile_pool(name="w_sb2", bufs=wb) as wt_pool,
                tc.tile_pool(name="ctx_sb", bufs=2) as ctx_sb_pool,
            ):
                import contextlib
                rep_ctx = tc.For_i(0, reps, 1) if reps > 1 else contextlib.nullcontext()
                prev_tail = [None]
                with rep_ctx:
                    for qb in range(NQB):
                        q0 = qb * QB
                        ctx_ps = ctx_ps_pool.tile([2 * DH, H, QB // 2], F32, tag="ctx")
                        ngrp = NTC // grp

                        def emit_stage_b(g, wt, ctx_ps=ctx_ps):
                            for ci in range(grp):
                                t = g * grp + ci
                                for sub in (0, 1):
                                    nc.tensor.matmul(
                                        ctx_ps[sub * DH:(sub + 1) * DH, :, :],
                                        lhsT=v_sb[:, t, :],
                                        rhs=wt[:, ci, :,
                                               sub * (QB // 2):(sub + 1) * (QB // 2)],
                                        start=(t == 0), stop=(t == NTC - 1),
                                        tile_position=(0, sub * DH),
                                        skip_group_check=True)

                        def emit_tail(qb_, ctx_ps_):
                            q0_ = qb_ * QB
                            ctx_sb = ctx_sb_pool.tile([2 * DH, H, QB // 2], F32)
                            if ctx_act:
                                nc.scalar.copy(ctx_sb, ctx_ps_)
                            else:
                                nc.vector.tensor_copy(ctx_sb, ctx_ps_)
                            o_ps = s_ps_pool.tile([DH, 2, H, DH], F32, tag="s")
                            for h in range(H):
                                for sub in (0, 1):
                                    nc.tensor.transpose(
                                        o_ps[:, sub, h, :],
                                        ctx_sb[sub * DH:(sub + 1) * DH, h, :],
                                        ident[sub * DH:(sub + 1) * DH,
                                              sub * DH:(sub + 1) * DH])
                            o_sb = ctx_sb_pool.tile([DH, 2, H, DH], F32, tag="o_sb")
                            if o_act:
                                nc.scalar.copy(o_sb, o_ps)
                            else:
                                nc.vector.tensor_copy(o_sb, o_ps)
                            out_ap = bass.AP(
                                tensor=out, offset=q0_ * D,
                                ap=[[D, DH], [DH * D, 2], [1, D]],
                            )
                            nc.sync.dma_start(out=out_ap, in_=o_sb)

                        pending = None  # (g, wt) whose stage-B is deferred
                        for g in range(ngrp):
                            e = e_pool.tile([TCH, grp, H, QB], BF16)
                            wt = wt_pool.tile([TCH, grp, H, QB], BF16)
                            for ci in range(grp):
                                t = g * grp + ci
                                s_ps = s_ps_pool.tile([TCH, H, QB], F32,
                                                      tag="s")
                                lhsT = vT_sb[:, t * TCH:(t + 1) * TCH]
                                nc.tensor.matmul(
                                    s_ps[:, 0:4, :], lhsT=mm_cast(lhsT),
                                    rhs=mm_cast(qwt_halves[0][:, :, q0:q0 + QB]),
                                    start=True, stop=True)
                                nc.tensor.matmul(
                                    s_ps[:, 4:8, :], lhsT=mm_cast(lhsT),
                                    rhs=mm_cast(qwt_halves[1][:, :, q0:q0 + QB]),
                                    start=True, stop=True)
                                nc.scalar.activation(
                                    out=e[:, ci, :, :], in_=s_ps,
                                    func=mybir.ActivationFunctionType.Exp,
                                    scale=0.125)

                            if g == tail_g and prev_tail[0] is not None:
                                emit_tail(*prev_tail[0])
                                prev_tail[0] = None
                            if pending is not None:
                                emit_stage_b(*pending)

                            # softmax over h, batched over the GRP chunks
                            gv = nc.gpsimd if gp_offload else nc.vector
                            t1 = tr_pool.tile([TCH, grp, 4, QB], BF16, tag="t1")
                            gv.tensor_add(t1, e[:, :, 0:4, :], e[:, :, 4:8, :])
                            t2 = tr_pool.tile([TCH, grp, 2, QB], BF16, tag="t2")
                            (nc.gpsimd if gp_l2 else nc.vector).tensor_add(
                                t2, t1[:, :, 0:2, :], t1[:, :, 2:4, :])
                            dsum = d_pool.tile([TCH, grp, QB], F32, tag="dsum")
                            nc.vector.tensor_add(dsum, t2[:, :, 0, :],
                                                 t2[:, :, 1, :])
                            r32 = d_pool.tile([TCH, grp, QB], F32, tag="r32")
                            nc.vector.reciprocal_approx_fast(out=r32, in_=dsum)
                            r16 = d_pool.tile([TCH, grp, QB], BF16, tag="r16")
                            (nc.gpsimd if gp_cast else nc.vector).tensor_copy(r16, r32)

                            r_bcast = bass.AP(
                                tensor=r16.tensor, offset=r16.offset,
                                ap=[r16.ap[0], r16.ap[1], [0, H], r16.ap[2]],
                            )
                            if gp_mulh:
                                r_bcast4 = bass.AP(
                                    tensor=r16.tensor, offset=r16.offset,
                                    ap=[r16.ap[0], r16.ap[1], [0, 4], r16.ap[2]],
                                )
                                nc.vector.tensor_mul(wt[:, :, 0:4, :],
                                                     e[:, :, 0:4, :], r_bcast4)
                                nc.gpsimd.tensor_mul(wt[:, :, 4:8, :],
                                                     e[:, :, 4:8, :], r_bcast4)
                            else:
                                nc.vector.tensor_mul(wt, e, r_bcast)
                            pending = (g, wt)

                        emit_stage_b(*pending)
                        prev_tail[0] = (qb, ctx_ps)
                    if prev_tail[0] is not None:
                        emit_tail(*prev_tail[0])
                        prev_tail[0] = None
    nc.compile()
    return nc


def _prep_inputs(query, value, conv_w, conv_b):
    """Build the 8 per-core input maps (host-side sharding + layout)."""
    sdt = np.float32 if SA_F32R else ml_dtypes.bfloat16
    W = conv_w[:, :, 0]  # [512, 64]
    w_aug = np.zeros((DH, H, DH + 1), np.float32)
    w_aug[:, :, :DH] = W.reshape(H, DH, DH).transpose(1, 0, 2)
    w_aug[:, :, DH] = conv_b.reshape(H, DH).T
    w_aug = w_aug.reshape(DH, H * (DH + 1)).astype(sdt)

    per_batch = {}
    for b in range(B):
        vT1 = np.concatenate(
            [value[b].T, np.ones((1, TV), np.float32)], axis=0
        ).astype(sdt)
        vPb = np.ascontiguousarray(
            value[b].reshape(NTC, TCH, DH).transpose(1, 0, 2)
        ).reshape(TCH, NTC * DH).astype(ml_dtypes.bfloat16)
        per_batch[b] = (np.ascontiguousarray(vT1), np.ascontiguousarray(vPb))

    in_maps = []
    for c in range(N_CORES):
        b = c // (N_CORES // B)
        qs = (c % (N_CORES // B)) * QPC
        # qTp[d, h, q] = query[b, qs+q, h*64+d]
        qTp = np.ascontiguousarray(
            query[b, qs:qs + QPC, :].reshape(QPC, H, DH).transpose(2, 1, 0)
        ).reshape(DH, H * QPC).astype(sdt)
        vT1, vPb = per_batch[b]
        in_maps.append({
            "qTp": np.ascontiguousarray(qTp),
            "vT1": vT1,
            "vP": vPb,
            "wA": w_aug,
        })
    return in_maps


def kernel(query, value, conv_w, conv_b, trace=False, **bench_kwargs):
    query = np.asarray(query, np.float32)
    value = np.asarray(value, np.float32)
    conv_w = np.asarray(conv_w, np.float32)
    conv_b = np.asarray(conv_b, np.float32)

    if "nc" not in _CACHE:
        _CACHE["nc"] = build_nc(**BUILD_CFG)
    nc = _CACHE["nc"]

    in_maps = _prep_inputs(query, value, conv_w, conv_b)
    res = run_bass_kernel_spmd(nc, in_maps, core_ids=list(range(N_CORES)),
                               trace=trace, **bench_kwargs)

    out = np.empty((B, TQ, D), np.float32)
    for c in range(N_CORES):
        b = c // (N_CORES // B)
        qs = (c % (N_CORES // B)) * QPC
        out[b, qs:qs + QPC, :] = res.results[c]["out"]
    if trace:
        return out, res
    return out
